# revision 1
# baseline (speedup 1.0000x reference)
"""HGCN embedding kernel for Trainium2 (8 NeuronCores, SPMD data-parallel).

Math: with the block-diagonal dense incidence (every batch's 32 nodes on all
8 hyperedges), B_inv = 1/32, D_inv = 1/8, and the propagation collapses to
    out[b, a] = mean_a'( input[b, a'] @ lin_w )          (same for all a)
so the whole module is
    y[b] = relu( mean_a(input[b,a,:]) @ (lin_w @ out_w) + hgcn_bias @ out_w + out_b )
    output[b, a, :] = y[b]
The device kernel streams input, reduces over the 32 agents with PE matmuls
against a block-ones matrix (which lands the mean already transposed [f, b]),
applies the folded weight + rank-1 bias matmul, ReLU, replicates x32 in SBUF
and streams the output back.
"""

import sys

import numpy as np

sys.path.insert(0, "/opt/trn_rl_repo")

BATCH = 4096
N_AG = 32
N_HE = 8
F_IN = 256
F_OUT = 128
NCORES = 8
BC = BATCH // NCORES          # 512 batches per core
GB = 128                      # batches per group (= SBUF partitions)
GROUPS = BC // GB             # 4
CHUNKS = 16                   # row-chunks of 128 rows per input tile
TILES = BC * N_AG // (CHUNKS * 128)   # 8 input tiles of [128, 16, 256] per core

_NC_CACHE = {}
TRACE = False
LAST_RESULT = None


BCAST_OUT_DMA = True


def _build_bass():
    import concourse.bacc as bacc
    import concourse.mybir as mybir
    import concourse.tile as tile
    from concourse.masks import make_identity

    f32 = mybir.dt.float32
    nc = bacc.Bacc("TRN2", target_bir_lowering=False, debug=False,
                   num_devices=NCORES)

    x = nc.declare_dram_parameter("x", [BC, N_AG, F_IN], f32, isOutput=False)
    w2 = nc.declare_dram_parameter("w2", [2, 128, F_OUT], f32, isOutput=False)
    cvec = nc.declare_dram_parameter("cvec", [1, F_OUT], f32, isOutput=False)
    ones1 = nc.declare_dram_parameter("ones1", [1, 128], f32, isOutput=False)
    out = nc.declare_dram_parameter("out", [BC, N_AG, F_OUT], f32, isOutput=True)

    xap = x.ap()
    outap = out.ap()

    with tile.TileContext(nc) as tc:
        with (
            tc.tile_pool(name="consts", bufs=1) as cpool,
            tc.tile_pool(name="xin", bufs=3) as xpool,
            tc.tile_pool(name="msum", bufs=2) as spool,
            tc.tile_pool(name="mt", bufs=4) as mpool,
            tc.tile_pool(name="rep", bufs=3) as rpool,
            tc.tile_pool(name="pt", bufs=4, space="PSUM") as ptpool,
            tc.tile_pool(name="py", bufs=2, space="PSUM") as pypool,
        ):
            w2t = cpool.tile([128, 2, F_OUT], f32)
            nc.scalar.dma_start(out=w2t[:], in_=w2.ap().rearrange("c p j -> p c j"))
            ct = cpool.tile([1, F_OUT], f32)
            nc.scalar.dma_start(out=ct[:], in_=cvec[:])
            o1 = cpool.tile([1, 128], f32)
            nc.scalar.dma_start(out=o1[:], in_=ones1[:])
            ident = cpool.tile([128, 128], f32)
            make_identity(nc, ident[:])

            for g in range(GROUPS):
                bt = xpool.tile([128, N_AG * F_IN], f32, tag="bt",
                                name=f"bt{g}")
                ieng = nc.sync if g % 2 == 0 else nc.gpsimd
                ieng.dma_start(
                    out=bt[:],
                    in_=xap[g * GB:(g + 1) * GB].rearrange("b a f -> b (a f)"))
                # mean over agents: in-place binary tree on the flat view;
                # every operand is dense step-1, all on DVE (GpSimd's
                # 2-input rate is ~5x worse and adds cross-engine stalls)
                S = N_AG * F_IN // 2
                while S >= F_IN:
                    nc.vector.tensor_add(
                        bt[:, 0:S], bt[:, 0:S], bt[:, S:2 * S])
                    S //= 2
                msb = bt[:, 0:F_IN]
                mts = []
                for fc in range(2):
                    pt = ptpool.tile([128, GB], f32, tag="pt", name=f"pt{g}_{fc}")
                    nc.tensor.transpose(
                        pt[:], msb[:, fc * 128:(fc + 1) * 128], ident[:])
                    mt = mpool.tile([128, GB], f32, tag="mt", name=f"mt{g}_{fc}")
                    nc.vector.tensor_copy(mt[:], pt[:])
                    mts.append(mt)
                py = pypool.tile([128, F_OUT], f32)
                for fc in range(2):
                    nc.tensor.matmul(py[:], mts[fc][:], w2t[:, fc, :],
                                     start=(fc == 0), stop=False)
                nc.tensor.matmul(py[:], o1[:], ct[:], start=False, stop=True)
                rep = rpool.tile([128, N_AG, F_OUT], f32, tag="rep",
                                 name=f"rep{g}")
                nc.scalar.activation(rep[:, 0, :], py[:],
                                     mybir.ActivationFunctionType.Relu)
                # last group: replication on DVE (free by then); else scalar
                ceng = nc.vector if g == GROUPS - 1 else nc.scalar
                cop = (nc.vector.tensor_copy if g == GROUPS - 1
                       else nc.scalar.copy)
                w = 1
                while w < N_AG // 2:
                    cop(rep[:, w:2 * w, :], rep[:, 0:w, :])
                    w *= 2
                # first half is complete; ship it while the last copy runs
                nc.scalar.dma_start(out=outap[g * GB:(g + 1) * GB, 0:w],
                                    in_=rep[:, 0:w, :])
                cop(rep[:, w:2 * w, :], rep[:, 0:w, :])
                nc.scalar.dma_start(out=outap[g * GB:(g + 1) * GB, w:2 * w],
                                    in_=rep[:, w:2 * w, :])
    nc.compile()
    return nc


def _get_nc():
    if "nc" not in _NC_CACHE:
        _NC_CACHE["nc"] = _build_bass()
    return _NC_CACHE["nc"]


def _is_block_pattern(node_idx, edge_idx):
    n = BATCH * N_AG * N_HE
    if node_idx.shape != (n,) or edge_idx.shape != (n,):
        return False
    i = np.arange(n, dtype=np.int64)
    if not np.array_equal(node_idx.astype(np.int64), i // N_HE):
        return False
    return np.array_equal(edge_idx.astype(np.int64),
                          (i // (N_AG * N_HE)) * N_HE + (i % N_HE))


def _fallback(inp, lin_w, hgcn_bias, out_w, out_b, node_idx, edge_idx):
    # general (host) path for arbitrary incidence — only used if the indices
    # are not the block-diagonal pattern produced by the reference setup
    n_nodes = BATCH * N_AG
    n_edges = BATCH * N_HE
    x = inp.reshape(-1, F_IN) @ lin_w
    node_idx = node_idx.astype(np.int64)
    edge_idx = edge_idx.astype(np.int64)
    D = np.bincount(node_idx, minlength=n_nodes).astype(np.float32)
    deg = np.bincount(edge_idx, minlength=n_edges).astype(np.float32)
    D_inv = np.where(D > 0, 1.0 / np.maximum(D, 1), 0.0).astype(np.float32)
    B_inv = np.where(deg > 0, 1.0 / np.maximum(deg, 1), 0.0).astype(np.float32)
    edge_feat = np.zeros((n_edges, F_OUT), np.float32)
    np.add.at(edge_feat, edge_idx, x[node_idx] * B_inv[edge_idx][:, None])
    outp = np.zeros((n_nodes, F_OUT), np.float32)
    np.add.at(outp, node_idx, edge_feat[edge_idx] * D_inv[node_idx][:, None])
    outp += hgcn_bias
    return np.maximum(outp @ out_w + out_b, 0.0)


def kernel(**inputs):
    global LAST_RESULT
    inp = np.ascontiguousarray(np.asarray(inputs["input"], np.float32))
    lin_w = np.asarray(inputs["lin_w"], np.float32)
    hgcn_bias = np.asarray(inputs["hgcn_bias"], np.float32)
    out_w = np.asarray(inputs["out_w"], np.float32)
    out_b = np.asarray(inputs["out_b"], np.float32)
    node_idx = np.asarray(inputs["node_idx"])
    edge_idx = np.asarray(inputs["edge_idx"])

    if not _is_block_pattern(node_idx, edge_idx):
        return _fallback(inp, lin_w, hgcn_bias, out_w, out_b,
                         node_idx, edge_idx)

    # fold: y = relu(mean_a(input) @ (lin_w @ out_w) + hgcn_bias @ out_w + out_b)
    w64 = lin_w.astype(np.float64) @ out_w.astype(np.float64)
    W = (w64 / N_AG).astype(np.float32)
    c = (hgcn_bias.astype(np.float64) @ out_w.astype(np.float64)
         + out_b).astype(np.float32)

    w2 = np.ascontiguousarray(W.reshape(2, 128, F_OUT))
    cvec = np.ascontiguousarray(c.reshape(1, F_OUT))
    ones1 = np.ones((1, 128), np.float32)

    from concourse.bass_utils import run_bass_kernel_spmd

    nc = _get_nc()
    in_maps = [
        {"x": inp[i * BC:(i + 1) * BC], "w2": w2, "cvec": cvec,
         "ones1": ones1}
        for i in range(NCORES)
    ]
    res = run_bass_kernel_spmd(nc, in_maps, list(range(NCORES)), trace=TRACE)
    LAST_RESULT = res
    full = np.concatenate([res.results[i]["out"] for i in range(NCORES)], axis=0)
    return full.reshape(BATCH * N_AG, F_OUT)



# revision 2
# speedup vs baseline: 2.6327x; 2.6327x over previous
"""HGCN embedding kernel for Trainium2 (8 NeuronCores, SPMD data-parallel).

Math: with the block-diagonal dense incidence (every batch's 32 nodes on all
8 hyperedges), B_inv = 1/32, D_inv = 1/8, and the propagation collapses to
    out[b, a] = mean_a'( input[b, a'] @ lin_w )          (same for all a)
so the whole module is
    y[b] = relu( mean_a(input[b,a,:]) @ (lin_w @ out_w) + hgcn_bias @ out_w + out_b )
    output[b, a, :] = y[b]

The device kernel is HBM-bandwidth bound, so all bulk traffic runs in bf16
(rel tolerance 2e-2 >> bf16 rounding).  Per core: 8 groups of 64 batches,
each a single contiguous 1 MB DMA laid out [128 partitions = 64 batches x
2 half-agent-blocks, 4096 free].  A 4-level in-place DVE tree (bf16 2x mode)
reduces the 16 agents within each partition; a PE matmul against a
pair-combine block matrix P2 sums the two partitions of each batch AND lands
the result transposed [feat, batch] in PSUM, ready for the folded-weight
matmul.  The bias enters as a rank-1 matmul, ReLU on the Act engine, and the
device ships only the 64x128 f32 unique rows per group; the host unshards by
broadcasting each row to the batch's 32 nodes.
"""

import sys

import numpy as np

sys.path.insert(0, "/opt/trn_rl_repo")

BATCH = 4096
N_AG = 32
N_HE = 8
F_IN = 256
F_OUT = 128
NCORES = 8
BC = BATCH // NCORES          # 512 batches per core
GB = 64                       # batches per group
NG = BC // GB                 # 8 groups per core
FREE = GB * N_AG * F_IN // 128   # 4096 bf16 elems per partition per group

_NC_CACHE = {}
TRACE = False
LAST_RESULT = None


def _build_bass():
    import concourse.bacc as bacc
    import concourse.mybir as mybir
    import concourse.tile as tile

    f32 = mybir.dt.float32
    bf16 = mybir.dt.bfloat16
    nc = bacc.Bacc("TRN2", target_bir_lowering=False, debug=False,
                   num_devices=NCORES)

    x = nc.declare_dram_parameter("x", [NG, 128, FREE], bf16, isOutput=False)
    w2 = nc.declare_dram_parameter("w2", [2, 128, F_OUT], bf16, isOutput=False)
    p2 = nc.declare_dram_parameter("p2", [128, GB], bf16, isOutput=False)
    cvec = nc.declare_dram_parameter("cvec", [1, F_OUT], bf16, isOutput=False)
    ones1 = nc.declare_dram_parameter("ones1", [1, GB], bf16, isOutput=False)
    out = nc.declare_dram_parameter("out", [BC, F_OUT], f32, isOutput=True)

    xap = x.ap()
    outap = out.ap()

    with tile.TileContext(nc) as tc:
        with (
            tc.tile_pool(name="consts", bufs=1) as cpool,
            tc.tile_pool(name="xin", bufs=4) as xpool,
            tc.tile_pool(name="mt", bufs=4) as mpool,
            tc.tile_pool(name="yt", bufs=3) as ypool,
            tc.tile_pool(name="pt", bufs=4, space="PSUM") as ptpool,
            tc.tile_pool(name="py", bufs=2, space="PSUM") as pypool,
        ):
            w2t = cpool.tile([128, 2, F_OUT], bf16)
            nc.scalar.dma_start(out=w2t[:], in_=w2.ap().rearrange("c p j -> p c j"))
            p2t = cpool.tile([128, GB], bf16)
            nc.scalar.dma_start(out=p2t[:], in_=p2[:])
            ct = cpool.tile([1, F_OUT], bf16)
            nc.scalar.dma_start(out=ct[:], in_=cvec[:])
            o1 = cpool.tile([1, GB], bf16)
            nc.scalar.dma_start(out=o1[:], in_=ones1[:])

            for g in range(NG):
                xg = xpool.tile([128, FREE], bf16, tag="xg", name=f"xg{g}")
                nc.sync.dma_start(out=xg[:], in_=xap[g])
                # reduce 16 agents per partition: in-place binary tree on the
                # flat bf16 view; dense step-1 operands hit DVE 2x mode
                S = FREE // 2
                while S >= F_IN:
                    nc.vector.tensor_add(
                        xg[:, 0:S], xg[:, 0:S], xg[:, S:2 * S])
                    S //= 2
                # pair-combine + transpose in one PE op: P2[p, b] = (p//2 == b)
                # sumsT[f, b] = sum_p xg[p, f] * P2[p, b]
                mts = []
                for fc in range(2):
                    pt = ptpool.tile([128, GB], f32, tag="pt", name=f"pt{g}_{fc}")
                    nc.tensor.matmul(pt[:], xg[:, fc * 128:(fc + 1) * 128],
                                     p2t[:], start=True, stop=True)
                    mt = mpool.tile([128, GB], bf16, tag="mt", name=f"mt{g}_{fc}")
                    nc.scalar.copy(mt[:], pt[:])
                    mts.append(mt)
                py = pypool.tile([GB, F_OUT], f32, tag="py", name=f"py{g}")
                for fc in range(2):
                    nc.tensor.matmul(py[:], mts[fc][:], w2t[:, fc, :],
                                     start=(fc == 0), stop=False)
                nc.tensor.matmul(py[:], o1[:], ct[:], start=False, stop=True)
                yt = ypool.tile([GB, F_OUT], f32, tag="yt", name=f"yt{g}")
                nc.scalar.activation(yt[:], py[:],
                                     mybir.ActivationFunctionType.Relu)
                nc.scalar.dma_start(out=outap[g * GB:(g + 1) * GB], in_=yt[:])
    nc.compile()
    return nc


def _get_nc():
    if "nc" not in _NC_CACHE:
        _NC_CACHE["nc"] = _build_bass()
    return _NC_CACHE["nc"]


def _is_block_pattern(node_idx, edge_idx):
    n = BATCH * N_AG * N_HE
    if node_idx.shape != (n,) or edge_idx.shape != (n,):
        return False
    i = np.arange(n, dtype=np.int64)
    if not np.array_equal(node_idx.astype(np.int64), i // N_HE):
        return False
    return np.array_equal(edge_idx.astype(np.int64),
                          (i // (N_AG * N_HE)) * N_HE + (i % N_HE))


def _fallback(inp, lin_w, hgcn_bias, out_w, out_b, node_idx, edge_idx):
    # general (host) path for arbitrary incidence — only used if the indices
    # are not the block-diagonal pattern produced by the reference setup
    n_nodes = BATCH * N_AG
    n_edges = BATCH * N_HE
    x = inp.reshape(-1, F_IN) @ lin_w
    node_idx = node_idx.astype(np.int64)
    edge_idx = edge_idx.astype(np.int64)
    D = np.bincount(node_idx, minlength=n_nodes).astype(np.float32)
    deg = np.bincount(edge_idx, minlength=n_edges).astype(np.float32)
    D_inv = np.where(D > 0, 1.0 / np.maximum(D, 1), 0.0).astype(np.float32)
    B_inv = np.where(deg > 0, 1.0 / np.maximum(deg, 1), 0.0).astype(np.float32)
    edge_feat = np.zeros((n_edges, F_OUT), np.float32)
    np.add.at(edge_feat, edge_idx, x[node_idx] * B_inv[edge_idx][:, None])
    outp = np.zeros((n_nodes, F_OUT), np.float32)
    np.add.at(outp, node_idx, edge_feat[edge_idx] * D_inv[node_idx][:, None])
    outp += hgcn_bias
    return np.maximum(outp @ out_w + out_b, 0.0)


def kernel(**inputs):
    global LAST_RESULT
    inp = np.ascontiguousarray(np.asarray(inputs["input"], np.float32))
    lin_w = np.asarray(inputs["lin_w"], np.float32)
    hgcn_bias = np.asarray(inputs["hgcn_bias"], np.float32)
    out_w = np.asarray(inputs["out_w"], np.float32)
    out_b = np.asarray(inputs["out_b"], np.float32)
    node_idx = np.asarray(inputs["node_idx"])
    edge_idx = np.asarray(inputs["edge_idx"])

    if not _is_block_pattern(node_idx, edge_idx):
        return _fallback(inp, lin_w, hgcn_bias, out_w, out_b,
                         node_idx, edge_idx)

    import ml_dtypes
    bf16 = ml_dtypes.bfloat16

    # fold: y = relu(mean_a(input) @ (lin_w @ out_w) + hgcn_bias @ out_w + out_b)
    w64 = lin_w.astype(np.float64) @ out_w.astype(np.float64)
    W = (w64 / N_AG).astype(bf16)
    c = (hgcn_bias.astype(np.float64) @ out_w.astype(np.float64)
         + out_b).astype(bf16)

    x16 = inp.astype(bf16)  # [BATCH, N_AG, F_IN]

    w2 = np.ascontiguousarray(W.reshape(2, 128, F_OUT))
    cvec = np.ascontiguousarray(c.reshape(1, F_OUT))
    ones1 = np.ones((1, GB), bf16)
    p2 = np.zeros((128, GB), bf16)
    p2[np.arange(128), np.arange(128) // 2] = 1

    from concourse.bass_utils import run_bass_kernel_spmd

    nc = _get_nc()
    in_maps = [
        {"x": x16[i * BC:(i + 1) * BC].reshape(NG, 128, FREE),
         "w2": w2, "p2": p2, "cvec": cvec, "ones1": ones1}
        for i in range(NCORES)
    ]
    res = run_bass_kernel_spmd(nc, in_maps, list(range(NCORES)), trace=TRACE)
    LAST_RESULT = res
    y = np.concatenate([res.results[i]["out"] for i in range(NCORES)], axis=0)
    # unshard: broadcast each batch's row back to its 32 identical node rows
    return np.repeat(np.asarray(y, np.float32), N_AG, axis=0)
